# revision 1
# baseline (speedup 1.0000x reference)
"""CNF dynamics kernel for Trainium2 (8 NeuronCores, data-parallel over batch).

Math (per batch row b):
    h[b,w]     = tanh(sum_d z[b,d] W[w,d] + B[w])
    dz[b,d]    = sum_w h[b,w] U[w,d] / WIDTH
    dlogp[b]   = -sum_w (1-h[b,w]^2) wu[w] / WIDTH,   wu[w] = sum_d W[w,d] U[w,d]

W/U/B depend only on the scalar t (tiny hypernet) -> computed once on host and
replicated to all cores. The big batch work runs on-device.

Device layout trick: the host hands each core z^T packed as [128, BC/2]
(two batch halves stacked on partitions: row h*64+d), so the contraction dim d
sits on partitions and every DMA is fully contiguous; block-diagonal weights
process both halves in one matmul. dz comes back in the same transposed layout
and the host untransposes during the gather.

dlogp is accumulated on-device entirely in PSUM via rotating-column lhsT
matmuls (s[b] = sum_w wu'[w] h^2) and finished on host as dlogp = s - sum(wu').
"""

import numpy as np

import concourse.bass as bass
import concourse.mybir as mybir
from concourse import bacc
from concourse.bass import ts
from concourse.tile import TileContext
from concourse.bass_utils import run_bass_kernel_spmd

D = 64
HID = 512
WIDTH = 64
BATCH = 524288
BLOCK = WIDTH * D
N_CORES = 8
BC = BATCH // N_CORES          # rows per core = 65536
HALF = BC // 2                 # columns of the packed z^T  = 32768
NT = 512                       # compute tile (free dim)
NG = 4096                      # dma tile (free dim)
N_TILES = HALF // NT           # 64
FP16 = mybir.dt.float16
FP32 = mybir.dt.float32

_nc_cache: dict = {}


def build_nc(repeat: int = 1):
    """Build + compile the per-core Bass program (cached per repeat count)."""
    if repeat in _nc_cache:
        return _nc_cache[repeat]

    nc = bacc.Bacc("TRN2", target_bir_lowering=False, debug=False)
    zT = nc.declare_dram_parameter("zT", [128, HALF], FP32, isOutput=False)
    wt_bd = nc.declare_dram_parameter("wt_bd", [128, 128], FP16, isOutput=False)
    u_bd = nc.declare_dram_parameter("u_bd", [128, 128], FP16, isOutput=False)
    wu_rot = nc.declare_dram_parameter("wu_rot", [128, 16, 32], FP16, isOutput=False)
    b2 = nc.declare_dram_parameter("b2", [128, 1], FP32, isOutput=False)
    dzT = nc.declare_dram_parameter("dzT", [128, HALF], FP32, isOutput=True)
    s_out = nc.declare_dram_parameter("s", [128, NT], FP32, isOutput=True)

    with TileContext(nc) as tc:
        with (
            tc.tile_pool(name="consts", bufs=1) as consts,
            tc.tile_pool(name="zin", bufs=3) as zin_pool,
            tc.tile_pool(name="dzo", bufs=3) as dzo_pool,
            tc.tile_pool(name="h", bufs=4) as h_pool,
            tc.tile_pool(name="hsq", bufs=4) as hsq_pool,
            tc.tile_pool(name="sacc", bufs=2) as sacc_pool,
            tc.tile_pool(name="ps_h", bufs=3, space=bass.MemorySpace.PSUM) as ps_h,
            tc.tile_pool(name="ps_dz", bufs=3, space=bass.MemorySpace.PSUM) as ps_dz,
            tc.tile_pool(name="ps_s", bufs=1, space=bass.MemorySpace.PSUM) as ps_s,
        ):
            wt_sb = consts.tile([128, 128], FP16)
            nc.sync.dma_start(out=wt_sb[:], in_=wt_bd[:])
            u_sb = consts.tile([128, 128], FP16)
            nc.sync.dma_start(out=u_sb[:], in_=u_bd[:])
            wr_sb = consts.tile([128, 16, 32], FP16)
            nc.sync.dma_start(out=wr_sb[:], in_=wu_rot[:])
            b2_sb = consts.tile([128, 1], FP32)
            nc.sync.dma_start(out=b2_sb[:], in_=b2[:])

            for _rep in range(repeat):
                strace = ps_s.tile([128, NT], FP32)
                for g in range(HALF // NG):
                    zin = zin_pool.tile([128, NG], FP16)
                    # SWDGE dma with inline fp32 -> fp16 cast
                    nc.gpsimd.dma_start(out=zin[:], in_=zT[:, ts(g, NG)])
                    dzo = dzo_pool.tile([128, NG], FP32)
                    for t in range(NG // NT):
                        i = g * (NG // NT) + t
                        hp = ps_h.tile([128, NT], FP32)
                        nc.tensor.matmul(
                            hp[:], wt_sb[:], zin[:, ts(t, NT)], start=True, stop=True
                        )
                        h = h_pool.tile([128, NT], FP16)
                        nc.scalar.activation(
                            h[:], hp[:], mybir.ActivationFunctionType.Tanh,
                            bias=b2_sb[:],
                        )
                        hsq = hsq_pool.tile([128, NT], FP16)
                        nc.vector.tensor_mul(hsq[:], h[:], h[:])
                        dzp = ps_dz.tile([128, NT], FP32)
                        nc.tensor.matmul(dzp[:], u_sb[:], h[:], start=True, stop=True)
                        j, r = i // 16, i % 16
                        nc.tensor.matmul(
                            strace[32 * j : 32 * j + 32, :],
                            wr_sb[:, r],
                            hsq[:],
                            start=(r == 0),
                            stop=(r == 15),
                            tile_position=(0, 32 * j),
                        )
                        # PSUM -> SBUF copy of dz, alternating engines
                        if i % 2 == 0:
                            nc.scalar.copy(dzo[:, ts(t, NT)], dzp[:])
                        else:
                            nc.vector.tensor_copy(dzo[:, ts(t, NT)], dzp[:])
                    nc.sync.dma_start(out=dzT[:, ts(g, NG)], in_=dzo[:])
                s_sb = sacc_pool.tile([128, NT], FP32)
                nc.vector.tensor_copy(s_sb[:], strace[:])
                nc.sync.dma_start(out=s_out[:], in_=s_sb[:])

    nc.compile()
    _nc_cache[repeat] = nc
    return nc


def prepare(t, z, logp_z, W1, b1, W2, b2, W3, b3):
    """Host-side hypernet + shard packing. Returns (in_maps, c0)."""
    t = np.asarray(t, np.float32)
    p = np.tanh(t.reshape(1, 1) @ W1 + b1)
    p = np.tanh(p @ W2 + b2)
    p = (p @ W3 + b3).reshape(-1).astype(np.float32)
    W = p[:BLOCK].reshape(WIDTH, D)
    U = p[BLOCK : 2 * BLOCK].reshape(WIDTH, D)
    G = p[2 * BLOCK : 3 * BLOCK].reshape(WIDTH, D)
    U = U * (1.0 / (1.0 + np.exp(-G)))
    B = p[3 * BLOCK :]
    wu = np.sum(W * U, axis=1) / WIDTH          # wu' = wu/WIDTH
    c0 = np.float32(wu.sum())
    Up = U / WIDTH

    bd = np.zeros((128, 128), np.float32)
    bd[:64, :64] = W.T
    bd[64:, 64:] = W.T
    wt_bd = bd.astype(np.float16)
    bd2 = np.zeros((128, 128), np.float32)
    bd2[:64, :64] = Up
    bd2[64:, 64:] = Up
    u_bd = bd2.astype(np.float16)

    wu_rot = np.zeros((128, 16, 32), np.float32)
    for r in range(16):
        wu_rot[0:64, r, 2 * r] = wu
        wu_rot[64:128, r, 2 * r + 1] = wu
    wu_rot = wu_rot.astype(np.float16)

    b2v = np.concatenate([B, B]).reshape(128, 1).astype(np.float32)

    # pack z -> per-core [128, HALF] transposed layout
    zq = np.ascontiguousarray(
        np.asarray(z, np.float32).reshape(N_CORES, 2, HALF, D).transpose(0, 1, 3, 2)
    ).reshape(N_CORES, 128, HALF)

    in_maps = [
        {
            "zT": zq[c],
            "wt_bd": wt_bd,
            "u_bd": u_bd,
            "wu_rot": wu_rot,
            "b2": b2v,
        }
        for c in range(N_CORES)
    ]
    return in_maps, c0


def postprocess(results, c0):
    dzT = np.stack([results[c]["dzT"] for c in range(N_CORES)])  # [8,128,HALF]
    dz = np.ascontiguousarray(
        dzT.reshape(N_CORES, 2, D, HALF).transpose(0, 1, 3, 2)
    ).reshape(BATCH, D)

    s = np.stack([results[c]["s"] for c in range(N_CORES)])      # [8,128,NT]
    # row p of s = trace for (j=p//32, r=(p%32)//2, half=p%2), tile i=16j+r,
    # batch-in-half = i*NT + n
    s = s.reshape(N_CORES, 4, 16, 2, NT).transpose(0, 3, 1, 2, 4).reshape(BATCH)
    dlogp = (s - c0).astype(np.float32).reshape(BATCH, 1)
    return dz, dlogp


def kernel(t, z, logp_z, W1, b1, W2, b2, W3, b3):
    nc = build_nc(1)
    in_maps, c0 = prepare(t, z, logp_z, W1, b1, W2, b2, W3, b3)
    res = run_bass_kernel_spmd(nc, in_maps, list(range(N_CORES)))
    return postprocess(res.results, c0)


# revision 5
# speedup vs baseline: 4781.7946x; 4781.7946x over previous
"""CNF dynamics kernel for Trainium2 (8 NeuronCores, data-parallel over batch).

Math (per batch row b):
    h[b,w]     = tanh(sum_d z[b,d] W[w,d] + B[w])
    dz[b,d]    = sum_w h[b,w] U[w,d] / WIDTH
    dlogp[b]   = -sum_w (1-h[b,w]^2) wu[w] / WIDTH,   wu[w] = sum_d W[w,d] U[w,d]

W/U/B depend only on the scalar t (tiny hypernet) -> computed once on host and
replicated to all cores. The big batch work runs on-device.

Device layout trick: the host hands each core z^T packed as [128, BC/2]
(two batch halves stacked on partitions: row h*64+d), so the contraction dim d
sits on partitions and every DMA is fully contiguous; block-diagonal weights
process both halves in one matmul. dz comes back in the same transposed layout
and the host untransposes during the gather.

dlogp is accumulated on-device entirely in PSUM via rotating-column lhsT
matmuls (s[b] = sum_w wu'[w] h^2) and finished on host as dlogp = s - sum(wu').
"""

import numpy as np

import jax
import concourse.bass as bass
import concourse.mybir as mybir
from concourse import bacc, bass2jax
from concourse.bass import ts
from concourse.tile import TileContext
from jax.experimental.shard_map import shard_map
from jax.sharding import Mesh, PartitionSpec

D = 64
HID = 512
WIDTH = 64
BATCH = 524288
BLOCK = WIDTH * D
N_CORES = 8
BC = BATCH // N_CORES          # rows per core = 65536
HALF = BC // 2                 # columns of the packed z^T  = 32768
NT = 512                       # compute tile (free dim)
NG = 4096                      # dma tile (free dim)
N_TILES = HALF // NT           # 64
FP16 = mybir.dt.float16
FP32 = mybir.dt.float32

_nc_cache: dict = {}


def build_nc(repeat: int = 1):
    """Build + compile the per-core Bass program (cached per repeat count)."""
    if repeat in _nc_cache:
        return _nc_cache[repeat]

    nc = bacc.Bacc("TRN2", target_bir_lowering=False, debug=False)
    zT = nc.declare_dram_parameter("zT", [128, HALF], FP32, isOutput=False)
    wt_bd = nc.declare_dram_parameter("wt_bd", [128, 128], FP16, isOutput=False)
    u_bd = nc.declare_dram_parameter("u_bd", [128, 128], FP16, isOutput=False)
    wu_rot = nc.declare_dram_parameter("wu_rot", [128, 16, 32], FP16, isOutput=False)
    b2 = nc.declare_dram_parameter("b2", [128, 1], FP32, isOutput=False)
    dzT = nc.declare_dram_parameter("dzT", [128, HALF], FP32, isOutput=True)
    s_out = nc.declare_dram_parameter("s", [128, NT], FP32, isOutput=True)

    with TileContext(nc) as tc:
        with (
            tc.tile_pool(name="consts", bufs=1) as consts,
            tc.tile_pool(name="zin", bufs=3) as zin_pool,
            tc.tile_pool(name="dzo", bufs=3) as dzo_pool,
            tc.tile_pool(name="h", bufs=4) as h_pool,
            tc.tile_pool(name="hsq", bufs=4) as hsq_pool,
            tc.tile_pool(name="sacc", bufs=2) as sacc_pool,
            tc.tile_pool(name="ps_h", bufs=3, space=bass.MemorySpace.PSUM) as ps_h,
            tc.tile_pool(name="ps_dz", bufs=3, space=bass.MemorySpace.PSUM) as ps_dz,
            tc.tile_pool(name="ps_s", bufs=1, space=bass.MemorySpace.PSUM) as ps_s,
        ):
            wt_sb = consts.tile([128, 128], FP16)
            nc.sync.dma_start(out=wt_sb[:], in_=wt_bd[:])
            u_sb = consts.tile([128, 128], FP16)
            nc.sync.dma_start(out=u_sb[:], in_=u_bd[:])
            wr_sb = consts.tile([128, 16, 32], FP16)
            nc.sync.dma_start(out=wr_sb[:], in_=wu_rot[:])
            b2_sb = consts.tile([128, 1], FP32)
            nc.sync.dma_start(out=b2_sb[:], in_=b2[:])

            import contextlib

            rep_ctx = (
                tc.For_i(0, repeat, 1, hint_engines=(mybir.EngineType.PE,))
                if repeat > 1
                else contextlib.nullcontext()
            )
            with rep_ctx:
                strace = ps_s.tile([128, NT], FP32)
                for g in range(HALF // NG):
                    zin = zin_pool.tile([128, NG], FP16)
                    # SWDGE dma with inline fp32 -> fp16 cast
                    nc.gpsimd.dma_start(out=zin[:], in_=zT[:, ts(g, NG)])
                    dzo = dzo_pool.tile([128, NG], FP32)
                    for t in range(NG // NT):
                        i = g * (NG // NT) + t
                        hp = ps_h.tile([128, NT], FP32)
                        nc.tensor.matmul(
                            hp[:], wt_sb[:], zin[:, ts(t, NT)], start=True, stop=True
                        )
                        h = h_pool.tile([128, NT], FP16)
                        nc.scalar.activation(
                            h[:], hp[:], mybir.ActivationFunctionType.Tanh,
                            bias=b2_sb[:],
                        )
                        hsq = hsq_pool.tile([128, NT], FP16)
                        nc.vector.tensor_mul(hsq[:], h[:], h[:])
                        dzp = ps_dz.tile([128, NT], FP32)
                        nc.tensor.matmul(dzp[:], u_sb[:], h[:], start=True, stop=True)
                        j, r = i // 16, i % 16
                        nc.tensor.matmul(
                            strace[32 * j : 32 * j + 32, :],
                            wr_sb[:, r],
                            hsq[:],
                            start=(r == 0),
                            stop=(r == 15),
                            tile_position=(0, 32 * j),
                        )
                        # PSUM -> SBUF copy of dz, alternating engines
                        if i % 2 == 0:
                            nc.scalar.copy(dzo[:, ts(t, NT)], dzp[:])
                        else:
                            nc.vector.tensor_copy(dzo[:, ts(t, NT)], dzp[:])
                    nc.sync.dma_start(out=dzT[:, ts(g, NG)], in_=dzo[:])
                s_sb = sacc_pool.tile([128, NT], FP32)
                nc.vector.tensor_copy(s_sb[:], strace[:])
                nc.sync.dma_start(out=s_out[:], in_=s_sb[:])

    nc.compile()
    _nc_cache[repeat] = nc
    return nc


def prepare(t, z, logp_z, W1, b1, W2, b2, W3, b3):
    """Host-side hypernet + shard packing. Returns (in_maps, c0)."""
    t = np.asarray(t, np.float32)
    p = np.tanh(t.reshape(1, 1) @ W1 + b1)
    p = np.tanh(p @ W2 + b2)
    p = (p @ W3 + b3).reshape(-1).astype(np.float32)
    W = p[:BLOCK].reshape(WIDTH, D)
    U = p[BLOCK : 2 * BLOCK].reshape(WIDTH, D)
    G = p[2 * BLOCK : 3 * BLOCK].reshape(WIDTH, D)
    U = U * (1.0 / (1.0 + np.exp(-G)))
    B = p[3 * BLOCK :]
    wu = np.sum(W * U, axis=1) / WIDTH          # wu' = wu/WIDTH
    c0 = np.float32(wu.sum())
    Up = U / WIDTH

    bd = np.zeros((128, 128), np.float32)
    bd[:64, :64] = W.T
    bd[64:, 64:] = W.T
    wt_bd = bd.astype(np.float16)
    bd2 = np.zeros((128, 128), np.float32)
    bd2[:64, :64] = Up
    bd2[64:, 64:] = Up
    u_bd = bd2.astype(np.float16)

    wu_rot = np.zeros((128, 16, 32), np.float32)
    for r in range(16):
        wu_rot[0:64, r, 2 * r] = wu
        wu_rot[64:128, r, 2 * r + 1] = wu
    wu_rot = wu_rot.astype(np.float16)

    b2v = np.concatenate([B, B]).reshape(128, 1).astype(np.float32)

    # pack z -> per-core [128, HALF] transposed layout
    zq = np.ascontiguousarray(
        np.asarray(z, np.float32).reshape(N_CORES, 2, HALF, D).transpose(0, 1, 3, 2)
    ).reshape(N_CORES, 128, HALF)

    in_maps = [
        {
            "zT": zq[c],
            "wt_bd": wt_bd,
            "u_bd": u_bd,
            "wu_rot": wu_rot,
            "b2": b2v,
        }
        for c in range(N_CORES)
    ]
    return in_maps, c0


def postprocess(results, c0):
    dzT = np.stack([results[c]["dzT"] for c in range(N_CORES)])  # [8,128,HALF]
    dz = np.ascontiguousarray(
        dzT.reshape(N_CORES, 2, D, HALF).transpose(0, 1, 3, 2)
    ).reshape(BATCH, D)

    s = np.stack([results[c]["s"] for c in range(N_CORES)])      # [8,128,NT]
    # row p of s = trace for (j=p//32, r=(p%32)//2, half=p%2), tile i=16j+r,
    # batch-in-half = i*NT + n
    s = s.reshape(N_CORES, 4, 16, 2, NT).transpose(0, 3, 1, 2, 4).reshape(BATCH)
    dlogp = (s - c0).astype(np.float32).reshape(BATCH, 1)
    return dz, dlogp


_runner_cache: dict = {}


def get_runner(repeat: int = 1):
    """Compile once; return (jitted_sharded_fn, meta). The jit is cached so
    repeated calls skip retracing/recompiling."""
    if repeat in _runner_cache:
        return _runner_cache[repeat]
    nc = build_nc(repeat)
    bass2jax.install_neuronx_cc_hook()
    partition_name = nc.partition_id_tensor.name if nc.partition_id_tensor else None
    in_names, out_names, out_avals = [], [], []
    for alloc in nc.m.functions[0].allocations:
        if not isinstance(alloc, mybir.MemoryLocationSet):
            continue
        name = alloc.memorylocations[0].name
        if alloc.kind == "ExternalInput":
            if name != partition_name:
                in_names.append(name)
        elif alloc.kind == "ExternalOutput":
            out_names.append(name)
            shape = tuple(alloc.tensor_shape)
            out_avals.append(jax.core.ShapedArray(shape, mybir.dt.np(alloc.dtype)))
    n_params = len(in_names)
    all_names = list(in_names) + list(out_names)
    if partition_name is not None:
        all_names.append(partition_name)
    all_names = tuple(all_names)
    donate = tuple(range(n_params, n_params + len(out_names)))

    def _body(*args):
        operands = list(args)
        if partition_name is not None:
            operands.append(bass2jax.partition_id_tensor())
        outs = bass2jax._bass_exec_p.bind(
            *operands,
            out_avals=tuple(out_avals),
            in_names=all_names,
            out_names=tuple(out_names),
            lowering_input_output_aliases=(),
            sim_require_finite=True,
            sim_require_nnan=True,
            nc=nc,
        )
        return tuple(outs)

    devices = jax.devices()[:N_CORES]
    mesh = Mesh(np.asarray(devices), ("core",))
    in_specs = (PartitionSpec("core"),) * (n_params + len(out_names))
    out_specs = (PartitionSpec("core"),) * len(out_names)
    sharded = jax.jit(
        shard_map(
            _body, mesh=mesh, in_specs=in_specs, out_specs=out_specs, check_rep=False
        ),
        donate_argnums=donate,
        keep_unused=True,
    )
    meta = dict(
        nc=nc, in_names=in_names, out_names=out_names, out_avals=out_avals,
        mesh=mesh, n_params=n_params,
    )
    _runner_cache[repeat] = (sharded, meta)
    return sharded, meta


def concat_inputs(in_maps, meta):
    return [
        np.concatenate([np.asarray(in_maps[c][nm]) for c in range(N_CORES)], axis=0)
        for nm in meta["in_names"]
    ]


def zero_outputs(meta):
    return [
        np.zeros((N_CORES * a.shape[0], *a.shape[1:]), a.dtype)
        for a in meta["out_avals"]
    ]


def split_outputs(out_arrs, meta):
    return [
        {
            nm: np.asarray(out_arrs[i]).reshape(
                N_CORES, *meta["out_avals"][i].shape
            )[c]
            for i, nm in enumerate(meta["out_names"])
        }
        for c in range(N_CORES)
    ]


def run_spmd(in_maps, repeat: int = 1):
    sharded, meta = get_runner(repeat)
    out_arrs = sharded(*concat_inputs(in_maps, meta), *zero_outputs(meta))
    return split_outputs(out_arrs, meta)


def kernel(t, z, logp_z, W1, b1, W2, b2, W3, b3):
    in_maps, c0 = prepare(t, z, logp_z, W1, b1, W2, b2, W3, b3)
    results = run_spmd(in_maps)
    return postprocess(results, c0)


# revision 10
# speedup vs baseline: 7861.4067x; 1.6440x over previous
"""CNF dynamics kernel for Trainium2 (8 NeuronCores, data-parallel over batch).

Math (per batch row b):
    h[b,w]     = tanh(sum_d z[b,d] W[w,d] + B[w])
    dz[b,d]    = sum_w h[b,w] U[w,d] / WIDTH
    dlogp[b]   = -sum_w (1-h[b,w]^2) wu[w] / WIDTH,   wu[w] = sum_d W[w,d] U[w,d]

W/U/B depend only on the scalar t (tiny hypernet) -> computed once on host and
replicated to all cores. The big batch work runs on-device.

Device layout trick: the host hands each core z^T packed as [128, BC/2]
(two batch halves stacked on partitions: row h*64+d), so the contraction dim d
sits on partitions and every DMA is fully contiguous; block-diagonal weights
process both halves in one matmul. dz comes back in the same transposed layout
and the host untransposes during the gather.

dlogp is accumulated on-device entirely in PSUM via rotating-column lhsT
matmuls (s[b] = sum_w wu'[w] h^2) and finished on host as dlogp = s - sum(wu').
"""

import numpy as np

import jax
import concourse.bass as bass
import concourse.mybir as mybir
from concourse import bacc, bass2jax
from concourse.bass import ts
from concourse.tile import TileContext
from jax.experimental.shard_map import shard_map
from jax.sharding import Mesh, PartitionSpec

D = 64
HID = 512
WIDTH = 64
BATCH = 524288
BLOCK = WIDTH * D
N_CORES = 8
BC = BATCH // N_CORES          # rows per core = 65536
HALF = BC // 2                 # columns of the packed z^T  = 32768
NT = 512                       # compute tile (free dim)
NG = 4096                      # dma tile (free dim)
N_TILES = HALF // NT           # 64
FP16 = mybir.dt.float16
FP32 = mybir.dt.float32

_nc_cache: dict = {}


def build_nc(repeat: int = 1):
    """Build + compile the per-core Bass program (cached per repeat count)."""
    if repeat in _nc_cache:
        return _nc_cache[repeat]

    nc = bacc.Bacc("TRN2", target_bir_lowering=False, debug=False)
    zT = nc.declare_dram_parameter("zT", [128, HALF], FP16, isOutput=False)
    wt_bd = nc.declare_dram_parameter("wt_bd", [128, 128], FP16, isOutput=False)
    u_bd = nc.declare_dram_parameter("u_bd", [128, 128], FP16, isOutput=False)
    wu_rot = nc.declare_dram_parameter("wu_rot", [128, 16, 32], FP16, isOutput=False)
    b2 = nc.declare_dram_parameter("b2", [128, 1], FP32, isOutput=False)
    dzT = nc.declare_dram_parameter("dzT", [128, HALF], FP16, isOutput=True)
    s_out = nc.declare_dram_parameter("s", [128, NT], FP32, isOutput=True)

    with TileContext(nc) as tc:
        with (
            tc.tile_pool(name="consts", bufs=1) as consts,
            tc.tile_pool(name="zin", bufs=3) as zin_pool,
            tc.tile_pool(name="dzo", bufs=3) as dzo_pool,
            tc.tile_pool(name="h", bufs=4) as h_pool,
            tc.tile_pool(name="hsq", bufs=4) as hsq_pool,
            tc.tile_pool(name="sacc", bufs=2) as sacc_pool,
            tc.tile_pool(name="ps_h", bufs=3, space=bass.MemorySpace.PSUM) as ps_h,
            tc.tile_pool(name="ps_dz", bufs=3, space=bass.MemorySpace.PSUM) as ps_dz,
            tc.tile_pool(name="ps_s", bufs=1, space=bass.MemorySpace.PSUM) as ps_s,
        ):
            wt_sb = consts.tile([128, 128], FP16)
            nc.sync.dma_start(out=wt_sb[:], in_=wt_bd[:])
            u_sb = consts.tile([128, 128], FP16)
            nc.sync.dma_start(out=u_sb[:], in_=u_bd[:])
            wr_sb = consts.tile([128, 16, 32], FP16)
            nc.sync.dma_start(out=wr_sb[:], in_=wu_rot[:])
            b2_sb = consts.tile([128, 1], FP32)
            nc.sync.dma_start(out=b2_sb[:], in_=b2[:])

            import contextlib

            rep_ctx = (
                tc.For_i(0, repeat, 1, hint_engines=(mybir.EngineType.PE,))
                if repeat > 1
                else contextlib.nullcontext()
            )
            with rep_ctx:
                strace = ps_s.tile([128, NT], FP32)
                for g in range(HALF // NG):
                    zin = zin_pool.tile([128, NG], FP16)
                    nc.sync.dma_start(out=zin[:], in_=zT[:, ts(g, NG)])
                    dzo = dzo_pool.tile([128, NG], FP16)
                    for t in range(NG // NT):
                        i = g * (NG // NT) + t
                        hp = ps_h.tile([128, NT], FP32)
                        nc.tensor.matmul(
                            hp[:], wt_sb[:], zin[:, ts(t, NT)], start=True, stop=True
                        )
                        h = h_pool.tile([128, NT], FP16)
                        nc.scalar.activation(
                            h[:], hp[:], mybir.ActivationFunctionType.Tanh,
                            bias=b2_sb[:],
                        )
                        hsq = hsq_pool.tile([128, NT], FP16)
                        nc.vector.tensor_mul(hsq[:], h[:], h[:])
                        dzp = ps_dz.tile([128, NT], FP32)
                        nc.tensor.matmul(dzp[:], u_sb[:], h[:], start=True, stop=True)
                        j, r = i // 16, i % 16
                        nc.tensor.matmul(
                            strace[32 * j : 32 * j + 32, :],
                            wr_sb[:, r],
                            hsq[:],
                            start=(r == 0),
                            stop=(r == 15),
                            tile_position=(0, 32 * j),
                        )
                        # PSUM -> SBUF copy of dz (fp32 psum -> fp16 sbuf),
                        # load-balanced ACT:DVE at 1:2
                        if i % 3 == 0:
                            nc.scalar.copy(dzo[:, ts(t, NT)], dzp[:])
                        else:
                            nc.vector.tensor_copy(dzo[:, ts(t, NT)], dzp[:])
                    nc.sync.dma_start(out=dzT[:, ts(g, NG)], in_=dzo[:])
                s_sb = sacc_pool.tile([128, NT], FP32)
                nc.vector.tensor_copy(s_sb[:], strace[:])
                nc.sync.dma_start(out=s_out[:], in_=s_sb[:])

    nc.compile()
    _nc_cache[repeat] = nc
    return nc


def prepare(t, z, logp_z, W1, b1, W2, b2, W3, b3):
    """Host-side hypernet + shard packing. Returns (in_maps, c0)."""
    t = np.asarray(t, np.float32)
    p = np.tanh(t.reshape(1, 1) @ W1 + b1)
    p = np.tanh(p @ W2 + b2)
    p = (p @ W3 + b3).reshape(-1).astype(np.float32)
    W = p[:BLOCK].reshape(WIDTH, D)
    U = p[BLOCK : 2 * BLOCK].reshape(WIDTH, D)
    G = p[2 * BLOCK : 3 * BLOCK].reshape(WIDTH, D)
    U = U * (1.0 / (1.0 + np.exp(-G)))
    B = p[3 * BLOCK :]
    wu = np.sum(W * U, axis=1) / WIDTH          # wu' = wu/WIDTH
    c0 = np.float32(wu.sum())
    Up = U / WIDTH

    bd = np.zeros((128, 128), np.float32)
    bd[:64, :64] = W.T
    bd[64:, 64:] = W.T
    wt_bd = bd.astype(np.float16)
    bd2 = np.zeros((128, 128), np.float32)
    bd2[:64, :64] = Up
    bd2[64:, 64:] = Up
    u_bd = bd2.astype(np.float16)

    wu_rot = np.zeros((128, 16, 32), np.float32)
    for r in range(16):
        wu_rot[0:64, r, 2 * r] = wu
        wu_rot[64:128, r, 2 * r + 1] = wu
    wu_rot = wu_rot.astype(np.float16)

    b2v = np.concatenate([B, B]).reshape(128, 1).astype(np.float32)

    # pack z -> per-core [128, HALF] transposed layout, fp16 on the wire
    zq = np.ascontiguousarray(
        np.asarray(z, np.float16).reshape(N_CORES, 2, HALF, D).transpose(0, 1, 3, 2)
    ).reshape(N_CORES, 128, HALF)

    in_maps = [
        {
            "zT": zq[c],
            "wt_bd": wt_bd,
            "u_bd": u_bd,
            "wu_rot": wu_rot,
            "b2": b2v,
        }
        for c in range(N_CORES)
    ]
    return in_maps, c0


def postprocess(results, c0):
    dzT = np.stack([results[c]["dzT"] for c in range(N_CORES)])  # [8,128,HALF]
    dz = np.ascontiguousarray(
        dzT.astype(np.float32).reshape(N_CORES, 2, D, HALF).transpose(0, 1, 3, 2)
    ).reshape(BATCH, D)

    s = np.stack([results[c]["s"] for c in range(N_CORES)])      # [8,128,NT]
    # row p of s = trace for (j=p//32, r=(p%32)//2, half=p%2), tile i=16j+r,
    # batch-in-half = i*NT + n
    s = s.reshape(N_CORES, 4, 16, 2, NT).transpose(0, 3, 1, 2, 4).reshape(BATCH)
    dlogp = (s - c0).astype(np.float32).reshape(BATCH, 1)
    return dz, dlogp


_runner_cache: dict = {}


def get_runner(repeat: int = 1):
    """Compile once; return (jitted_sharded_fn, meta). The jit is cached so
    repeated calls skip retracing/recompiling."""
    if repeat in _runner_cache:
        return _runner_cache[repeat]
    nc = build_nc(repeat)
    bass2jax.install_neuronx_cc_hook()
    partition_name = nc.partition_id_tensor.name if nc.partition_id_tensor else None
    in_names, out_names, out_avals = [], [], []
    for alloc in nc.m.functions[0].allocations:
        if not isinstance(alloc, mybir.MemoryLocationSet):
            continue
        name = alloc.memorylocations[0].name
        if alloc.kind == "ExternalInput":
            if name != partition_name:
                in_names.append(name)
        elif alloc.kind == "ExternalOutput":
            out_names.append(name)
            shape = tuple(alloc.tensor_shape)
            out_avals.append(jax.core.ShapedArray(shape, mybir.dt.np(alloc.dtype)))
    n_params = len(in_names)
    all_names = list(in_names) + list(out_names)
    if partition_name is not None:
        all_names.append(partition_name)
    all_names = tuple(all_names)
    donate = tuple(range(n_params, n_params + len(out_names)))

    def _body(*args):
        operands = list(args)
        if partition_name is not None:
            operands.append(bass2jax.partition_id_tensor())
        outs = bass2jax._bass_exec_p.bind(
            *operands,
            out_avals=tuple(out_avals),
            in_names=all_names,
            out_names=tuple(out_names),
            lowering_input_output_aliases=(),
            sim_require_finite=True,
            sim_require_nnan=True,
            nc=nc,
        )
        return tuple(outs)

    devices = jax.devices()[:N_CORES]
    mesh = Mesh(np.asarray(devices), ("core",))
    in_specs = (PartitionSpec("core"),) * (n_params + len(out_names))
    out_specs = (PartitionSpec("core"),) * len(out_names)
    sharded = jax.jit(
        shard_map(
            _body, mesh=mesh, in_specs=in_specs, out_specs=out_specs, check_rep=False
        ),
        donate_argnums=donate,
        keep_unused=True,
    )
    meta = dict(
        nc=nc, in_names=in_names, out_names=out_names, out_avals=out_avals,
        mesh=mesh, n_params=n_params,
    )
    _runner_cache[repeat] = (sharded, meta)
    return sharded, meta


def concat_inputs(in_maps, meta):
    return [
        np.concatenate([np.asarray(in_maps[c][nm]) for c in range(N_CORES)], axis=0)
        for nm in meta["in_names"]
    ]


def zero_outputs(meta):
    return [
        np.zeros((N_CORES * a.shape[0], *a.shape[1:]), a.dtype)
        for a in meta["out_avals"]
    ]


def split_outputs(out_arrs, meta):
    return [
        {
            nm: np.asarray(out_arrs[i]).reshape(
                N_CORES, *meta["out_avals"][i].shape
            )[c]
            for i, nm in enumerate(meta["out_names"])
        }
        for c in range(N_CORES)
    ]


def run_spmd(in_maps, repeat: int = 1):
    sharded, meta = get_runner(repeat)
    out_arrs = sharded(*concat_inputs(in_maps, meta), *zero_outputs(meta))
    return split_outputs(out_arrs, meta)


def kernel(t, z, logp_z, W1, b1, W2, b2, W3, b3):
    in_maps, c0 = prepare(t, z, logp_z, W1, b1, W2, b2, W3, b3)
    results = run_spmd(in_maps)
    return postprocess(results, c0)
